# revision 8
# baseline (speedup 1.0000x reference)
"""Cross-attention kernel for TRN2, 8 NeuronCores, data-parallel over batch.

Problem (per full input):
    h_enc: [16, 2048, 1024] f32, h_dec: [512, 16, 1024] f32
    e[b,:,:] = h_enc[b] @ h_dec[:,b,:].T          # [T_enc, T_dec]
    a = softmax(e, axis=T_enc)
    c[b] = a.T @ h_enc[b]                         # [T_dec, D]

Sharding: B=16 -> 2 batches per core (embarrassingly parallel, no
collectives). Each core computes its 2 batches; host concatenates.

Per-core plan (fp16 compute on the PE, f32 PSUM accumulation — fp16 is
1 cycle/row like bf16 but with an 11-bit mantissa, which the softmax
logits need):
  - inputs enter as f32 HWDGE loads in 256-row (1MB) blocks, split
    across BOTH HWDGE queues (scalar + sync) so batch 0 lands in ~17us.
    NO SWDGE (gpsimd) DMAs anywhere: the xbar DMA-transposes and SWDGE
    DMAs mutually serialize (deadlock guard), which ping-pongs the whole
    input phase (measured v3: 47us tensor stall).
  - f32 -> fp16 casts on compute engines: DVE for batch 0, the
    otherwise-idle GpSimd for batch 1 (keeps the DVE FIFO free for
    batch-0 softmax work).
  - heT/hdT d-major tiles via merged xbar DMA-transposes (one per
    512-row pair of cast blocks, [128, 4096] fp16, canonical
    [128, B, 128] dest form) on the sync queue.
  - 32 warmup matmuls at t=0 trip the PE HAM clock gate to 2.4GHz.
  - per (batch, t-tile) stage, software-pipelined with its predecessor:
      matmul1: S[128, 2048] += hdT.T @ heT  (8 K-chunks x 4 N-chunks)
      softmax over the free axis: DVE chunked reduce_max(negate) -> ACT
        exp(S+bias) with fused accum_out rowsum -> DVE reciprocal
      P^T via ONE merged xbar transpose ([128, 2048] fp16)
      matmul2: C[128, 1024] += PT.T @ he_nat  (16 K-chunks x 2 N-chunks)
      normalize by 1/rowsum (DVE tensor_scalar_mul), store f32 via the
      scalar-engine HWDGE queue
"""

import numpy as np

import bass_rust
import concourse.bass as bass
import concourse.mybir as mybir
import concourse.tile as tile
from concourse.bass_utils import run_bass_kernel_spmd
from concourse.masks import make_identity

FP16 = mybir.dt.float16
F32 = mybir.dt.float32

B_FULL = 16
N_CORES = 8
B_PER_CORE = B_FULL // N_CORES  # 2
T_ENC = 2048
T_DEC = 512
D = 1024
P = 128
E_CHUNKS = T_ENC // P  # 16
D_CHUNKS = D // P      # 8
T_CHUNKS = T_DEC // P  # 4
N1 = 512               # matmul1 N tile (one PSUM bank)
N2 = 512               # matmul2 N tile
SB = 256               # load block rows (2 chunks, 1MB f32)
N_SB_HE = T_ENC // SB  # 8 per batch
N_SB_HD = T_DEC // SB  # 2 per batch
WB = 512               # transpose block (T_enc rows) = 2 load blocks
N_WB = T_ENC // WB     # 4


def split_excess_waits(nc, max_waits: int = 1):
    """This toolchain's walrus accepts only ONE sync-wait command per
    instruction (setupSyncWait raises "Too many sync wait commands"), but
    Tile attaches one wait per producing proc. Hoist excess waits onto
    same-engine NOP carriers inserted just before the instruction."""
    for fn in nc.m.functions:
        for blk in fn.blocks:
            insts = list(blk.instructions)
            new_list = []
            changed = False
            for inst in insts:
                si = inst.sync_info
                waits = list(si.on_wait) if si is not None else []
                if len(waits) > max_waits:
                    changed = True
                    for j, w in enumerate(waits[max_waits:]):
                        nop = mybir.InstNoOp(
                            name=f"{inst.name}-wc{j}",
                            engine=inst.engine,
                            bass_nofuse=True,
                            sync_info=mybir.SyncInfo(on_wait=[w], on_update=[]),
                        )
                        new_list.append(nop)
                    inst.sync_info = bass_rust.SyncInfo(
                        on_wait=waits[:max_waits], on_update=list(si.on_update)
                    )
                new_list.append(inst)
            if changed:
                blk.instructions = new_list


def build_attention_core():
    nc = bass.Bass("TRN2", target_bir_lowering=False, dynamic_dma_scratch_size=1024)
    h_enc = nc.declare_dram_parameter(
        "h_enc", [B_PER_CORE, T_ENC, D], F32, isOutput=False
    )
    h_dec = nc.declare_dram_parameter(
        "h_dec", [T_DEC, B_PER_CORE, D], F32, isOutput=False
    )
    out = nc.declare_dram_parameter(
        "out", [B_PER_CORE, T_DEC, D], F32, isOutput=True
    )

    with tile.TileContext(nc) as tc:
        with (
            tc.tile_pool(name="singles", bufs=1) as singles_pool,
            tc.tile_pool(name="stage", bufs=3) as stage_pool,
            tc.tile_pool(name="p", bufs=2) as p_pool,
            tc.tile_pool(name="pt", bufs=2) as pt_pool,
            tc.tile_pool(name="c", bufs=1) as c_pool,
            tc.tile_pool(name="stats", bufs=4) as stats_pool,
            tc.tile_pool(name="psum_s", bufs=1, space="PSUM") as psum_s_pool,
            tc.tile_pool(name="psum_c", bufs=2, space="PSUM") as psum_c_pool,
        ):
            identity = singles_pool.tile([P, P], FP16)
            make_identity(nc, identity)

            # dedicated per-batch input tiles (no pool rotation)
            # he_nat[p=te_low, ec, d];  heT[p=d_low, ec, dc*128+te_low]
            he_nat = [
                singles_pool.tile([P, E_CHUNKS, D], FP16, name=f"he_nat{b}")
                for b in range(B_PER_CORE)
            ]
            heT = [
                singles_pool.tile([P, E_CHUNKS, D], FP16, name=f"heT{b}")
                for b in range(B_PER_CORE)
            ]
            # hd_nat[p=td_low, tc, d];  hdT[p=d_low, tc, dc*128+td_low]
            hd_nat = [
                singles_pool.tile([P, T_CHUNKS, D], FP16, name=f"hd_nat{b}")
                for b in range(B_PER_CORE)
            ]
            hdT = [
                singles_pool.tile([P, T_CHUNKS, D], FP16, name=f"hdT{b}")
                for b in range(B_PER_CORE)
            ]

            # ---- PE warmup: trip the HAM clock gate to 2.4GHz while the
            # input DMAs stream in. Writes the s_psum buffer; stage (0,0)
            # takes a WAW dep on it, long satisfied by the time its
            # inputs land.
            warm_psum = psum_s_pool.tile([P, T_ENC], F32, tag="s_psum")
            for _ in range(32):
                nc.tensor.matmul(
                    warm_psum[:, :P], lhsT=identity, rhs=identity,
                    start=True, stop=True,
                )

            def load_cast(dma_engine, cast_engine, dst, src_rows):
                """f32 HWDGE load of a [SB=256, D] DRAM slice into a stage
                tile, then engine cast into dst ([128, 2, 1024] fp16)."""
                stage = stage_pool.tile([P, SB // P, D], F32, tag="stage")
                src = src_rows.rearrange("(c p) d -> p c d", p=P)
                dma_engine.dma_start(out=stage, in_=src)
                cast_engine.tensor_copy(dst, stage)

            # ---- batch-0 inputs: loads split across BOTH HWDGE queues,
            # casts on DVE. h_dec first (it gates matmul1's lhsT).
            for hb in range(N_SB_HD):
                load_cast(
                    nc.scalar, nc.vector,
                    hd_nat[0][:, 2 * hb : 2 * hb + 2, :],
                    h_dec.ap()[hb * SB : (hb + 1) * SB, 0, :],
                )
            for sb in range(N_SB_HE):
                eng = nc.scalar if sb % 2 == 0 else nc.sync
                load_cast(
                    eng, nc.vector,
                    he_nat[0][:, 2 * sb : 2 * sb + 2, :],
                    h_enc.ap()[0, sb * SB : (sb + 1) * SB, :],
                )
            # ---- batch-1 inputs: loads on the scalar queue (sync queue
            # is running batch-0 transposes by then), casts on GpSimd so
            # the DVE FIFO stays free for batch-0 softmax work.
            for hb in range(N_SB_HD):
                load_cast(
                    nc.scalar, nc.gpsimd,
                    hd_nat[1][:, 2 * hb : 2 * hb + 2, :],
                    h_dec.ap()[hb * SB : (hb + 1) * SB, 1, :],
                )
            for sb in range(N_SB_HE):
                load_cast(
                    nc.scalar, nc.gpsimd,
                    he_nat[1][:, 2 * sb : 2 * sb + 2, :],
                    h_enc.ap()[1, sb * SB : (sb + 1) * SB, :],
                )

            def emit_input_transposes(b):
                """Merged xbar transposes on the sync queue: one per
                512-row block. The dest AP must be the canonical
                [128, B, 128] block form: out[p, blk, c] = in_[c, blk*128+p]."""
                nc.sync.dma_start(
                    out=hdT[b].rearrange("p e (k c) -> p (e k) c", c=P),
                    in_=hd_nat[b],
                    transpose=True,
                )
                for wb in range(N_WB):
                    nc.sync.dma_start(
                        out=heT[b][:, 4 * wb : 4 * wb + 4, :].rearrange(
                            "p e (k c) -> p (e k) c", c=P
                        ),
                        in_=he_nat[b][:, 4 * wb : 4 * wb + 4, :],
                        transpose=True,
                    )

            emit_input_transposes(0)

            def emit_pt(stage):
                """P^T via one merged xbar transpose -> pt[p=te_low, ec, td]."""
                b, m, p_tile, recip = stage
                pt_tile = pt_pool.tile([P, E_CHUNKS, P], FP16, tag="pt")
                nc.sync.dma_start(out=pt_tile, in_=p_tile, transpose=True)
                return pt_tile

            def emit_mm2(stage, pt_tile):
                b, m, p_tile, recip = stage
                m_sl = slice(m * P, (m + 1) * P)
                c_psum = psum_c_pool.tile([P, D], F32, tag="c_psum")
                for ko in range(E_CHUNKS):
                    for no in range(D // N2):
                        nc.tensor.matmul(
                            c_psum[:, no * N2 : (no + 1) * N2],
                            lhsT=pt_tile[:, ko, :],
                            rhs=he_nat[b][:, ko, no * N2 : (no + 1) * N2],
                            start=(ko == 0),
                            stop=(ko == E_CHUNKS - 1),
                        )
                c_sbuf = c_pool.tile([P, D], F32, tag="c")
                nc.vector.tensor_scalar_mul(c_sbuf, c_psum, recip)
                nc.scalar.dma_start(out=out.ap()[b, m_sl, :], in_=c_sbuf)

            prev = None
            for b in range(B_PER_CORE):
                for m in range(T_CHUNKS):
                    # PT of the previous stage first: the xbar transpose
                    # runs during this stage's matmul1, keeping mm2(prev)
                    # fed.
                    pt_prev = emit_pt(prev) if prev is not None else None
                    if b == 0 and m == 2:
                        # batch-1 sync-queue transposes between PT(0,1)
                        # and PT(0,2): late enough not to head-of-line
                        # block batch 0's PTs, early enough to be done
                        # before stage (1,0).
                        emit_input_transposes(1)

                    # ---- matmul1: S = h_dec_tile @ h_enc.T ----
                    s_psum = psum_s_pool.tile([P, T_ENC], F32, tag="s_psum")
                    for no in range(T_ENC // N1):
                        for ko in range(D_CHUNKS):
                            nc.tensor.matmul(
                                s_psum[:, no * N1 : (no + 1) * N1],
                                lhsT=hdT[b][:, m, ko * P : (ko + 1) * P],
                                rhs=heT[b][
                                    :, 4 * no : 4 * no + 4, ko * P : (ko + 1) * P
                                ],
                                start=(ko == 0),
                                stop=(ko == D_CHUNKS - 1),
                            )

                    # ---- softmax over free axis (T_enc) ----
                    # chunked max: partials per 512-column group (overlap
                    # with the remaining matmuls), tiny combine at the end
                    pmax = stats_pool.tile([P, 4], F32, tag="pmax")
                    for no in range(4):
                        nc.vector.tensor_reduce(
                            out=pmax[:, no : no + 1],
                            in_=s_psum[:, no * N1 : (no + 1) * N1],
                            axis=mybir.AxisListType.X,
                            op=mybir.AluOpType.max,
                        )
                    negmax = stats_pool.tile([P, 1], F32, tag="negmax")
                    nc.vector.tensor_reduce(
                        out=negmax,
                        in_=pmax,
                        axis=mybir.AxisListType.X,
                        op=mybir.AluOpType.max,
                        negate=True,
                    )
                    p_tile = p_pool.tile([P, T_ENC], FP16, tag="p")
                    rowsum = stats_pool.tile([P, 1], F32, tag="rowsum")
                    nc.scalar.activation(
                        out=p_tile,
                        in_=s_psum,
                        func=mybir.ActivationFunctionType.Exp,
                        bias=negmax,
                        scale=1.0,
                        accum_out=rowsum,
                    )
                    recip = stats_pool.tile([P, 1], F32, tag="recip")
                    nc.vector.reciprocal(recip, rowsum)

                    # ---- finish the previous stage ----
                    if prev is not None:
                        emit_mm2(prev, pt_prev)
                    prev = (b, m, p_tile, recip)

            pt_prev = emit_pt(prev)
            emit_mm2(prev, pt_prev)

    split_excess_waits(nc)
    return nc


_NC_CACHE = None


def _get_nc():
    global _NC_CACHE
    if _NC_CACHE is None:
        _NC_CACHE = build_attention_core()
    return _NC_CACHE


def kernel(**inputs) -> np.ndarray:
    h_enc = np.ascontiguousarray(np.asarray(inputs["h_enc"], dtype=np.float32))
    h_dec = np.ascontiguousarray(np.asarray(inputs["h_dec"], dtype=np.float32))
    assert h_enc.shape == (B_FULL, T_ENC, D)
    assert h_dec.shape == (T_DEC, B_FULL, D)

    nc = _get_nc()
    in_maps = []
    for i in range(N_CORES):
        sl = slice(i * B_PER_CORE, (i + 1) * B_PER_CORE)
        in_maps.append(
            {
                "h_enc": np.ascontiguousarray(h_enc[sl]),
                "h_dec": np.ascontiguousarray(h_dec[:, sl, :]),
            }
        )
    res = run_bass_kernel_spmd(nc, in_maps, core_ids=list(range(N_CORES)))
    out = np.concatenate([res.results[i]["out"] for i in range(N_CORES)], axis=0)
    return np.ascontiguousarray(out.astype(np.float32))
